# revision 1
# baseline (speedup 1.0000x reference)
"""ArcFace loss (B=1024, D=512, C=50000) distributed over 8 TRN2 NeuronCores.

Strategy (classification/tensor parallel, per the sharding hint):
  - weight [50000, 512] split along classes: 6250/core padded to 6272 = 49*128
    (pad rows are zero; each contributes exp(0)=1 to the sum and a constant
    176 is subtracted globally).  The host stages the shard twice in bf16
    (scaled by 16 for fp8 headroom): row-major [CP, D] for the per-class
    norms + label gather, and transposed [128, KC, CP]; the transposed copy
    streams HBM->SBUF through a casting SWDGE DMA into fp8e4 lhsT tiles.
  - embeddings are replicated and staged twice: row-major f32 (norms + target
    path) and raw-transposed bf16 [128, KC, B].  Each core computes
    16/||e_b|| on DVE, round-trips it through DRAM and an indirect-gather
    (index 0) to replicate it across partitions, then one DVE multiply per
    contraction chunk produces the normalized fp8 moving operand - no
    PE transposes in the prologue.
  - per class tile: fp8 DoubleRow TensorE matmuls (two 256-deep passes)
    accumulate raw logits into two [128, 512] FP32 PSUM banks; ScalarE
    computes exp(raw * 4*rsqrt(ssq)) with the per-class norm folded in as the
    per-partition activation scale.  ssq comes from a fused DVE bf16
    multiply+row-sum over the row chunks; the Newton-rsqrt scale pipeline
    runs two chunks ahead so exp never waits.
  - exp tiles accumulate into two alternating bf16 accumulators on DVE,
    reduced over class partitions via PE-transpose + DVE row reduction.
  - the masked target-class terms (gather w16[label] rows by indirect DMA,
    cos_t/phi/exp corrections in f32) and the partial sums S go through one
    12 KB AllReduce; every core redundantly finishes Z = S + delta - 176,
    nll = ln Z - 64*phi, loss = mean(nll).
"""

import numpy as np

try:
    import concourse.bass as bass  # noqa: F401
except ImportError:  # fallback when PYTHONPATH lacks the repo
    import sys

    for p in ("/opt/trn_rl_repo", "/root/.axon_site/_ro/trn_rl_repo"):
        sys.path.insert(0, p)
    import concourse.bass as bass  # noqa: F401

import concourse.bacc as bacc
import concourse.tile as tile
from concourse import mybir
from concourse.bass_utils import run_bass_kernel_spmd
from concourse.masks import make_identity

F32 = mybir.dt.float32
BF16 = mybir.dt.bfloat16
FP8 = mybir.dt.float8e4
I32 = mybir.dt.int32
AF = mybir.ActivationFunctionType
ALU = mybir.AluOpType
AX = mybir.AxisListType
DR = mybir.MatmulPerfMode.DoubleRow

B, D, C = 1024, 512, 50000
NCORES = 8
CS = C // NCORES          # 6250 real classes per core
NT = 49                   # class tiles of 128 per core
CP = NT * 128             # 6272 padded classes per core
KC = D // 128             # 4 contraction chunks
G = B // 128              # 8 batch groups of 128
CH = 7                    # tiles per prefetch/scale chunk (NT = 7*7)
NCH = NT // CH
PADS = float(NCORES * (CP - CS))  # zero pad rows -> exp(0)=1 each

SCALE = 64.0
MARGIN = 0.5
COS_M = float(np.cos(MARGIN))
SIN_M = float(np.sin(MARGIN))
TH = float(np.cos(np.pi - MARGIN))
MM = float(np.sin(np.pi - MARGIN) * MARGIN)

_CACHED_NC = None


def _newton_rsqrt(nc, pool, s_ap, out_ap, n, iters=2, name="nr", bufs=4):
    """out = 1/sqrt(s) elementwise on DVE via bit-trick seed + Newton."""
    j = pool.tile([128, n], I32, tag=f"{name}_j", bufs=bufs, name=f"{name}_j")
    nc.vector.tensor_scalar(j[:], s_ap.bitcast(I32), 1, None,
                            op0=ALU.arith_shift_right)
    nc.vector.tensor_scalar(j[:], j[:], -1, 0x5F3759DF, op0=ALU.mult, op1=ALU.add)
    r = pool.tile([128, n], F32, tag=f"{name}_r", bufs=bufs, name=f"{name}_r")
    nc.vector.tensor_copy(r[:].bitcast(I32), j[:])
    t = pool.tile([128, n], F32, tag=f"{name}_t", bufs=bufs, name=f"{name}_t")
    for _ in range(iters):
        nc.vector.tensor_mul(t[:], r[:], r[:])
        nc.vector.tensor_mul(t[:], t[:], s_ap)
        nc.vector.tensor_scalar(t[:], t[:], -0.5, 1.5, op0=ALU.mult, op1=ALU.add)
        nc.vector.tensor_mul(r[:], r[:], t[:])
    nc.vector.tensor_copy(out_ap, r[:])


def build_kernel():
    nc = bacc.Bacc("TRN2", target_bir_lowering=False, num_devices=NCORES)

    emb_d = nc.declare_dram_parameter("emb", [B, D], F32, isOutput=False)
    eT16_d = nc.declare_dram_parameter("eT16", [128, KC, B], BF16, isOutput=False)
    w16_d = nc.declare_dram_parameter("w16", [CP, D], BF16, isOutput=False)
    wT16_d = nc.declare_dram_parameter("wT16", [128, KC, CP], BF16, isOutput=False)
    lblg_d = nc.declare_dram_parameter("lblg", [128, G], I32, isOutput=False)
    coff_d = nc.declare_dram_parameter("coff", [128, 1], F32, isOutput=False)
    out_d = nc.declare_dram_parameter("out", [1, 1], F32, isOutput=True)

    with tile.TileContext(nc) as tc:
        with (
            tc.tile_pool(name="const", bufs=1) as cpool,
            tc.tile_pool(name="big", bufs=1) as big,
            tc.tile_pool(name="wrow", bufs=4) as wrow,
            tc.tile_pool(name="wtp", bufs=4) as wtp,
            tc.tile_pool(name="nr", bufs=4) as nrp,
            tc.tile_pool(name="sc", bufs=4) as scp,
            tc.tile_pool(name="ex", bufs=4) as exp_pool,
            tc.tile_pool(name="tgt", bufs=1) as tgt,
            tc.tile_pool(name="pt_ps", bufs=1, space="PSUM") as pt_ps,
            tc.tile_pool(name="cos_ps", bufs=2, space="PSUM") as cos_ps,
            tc.tile_pool(name="fin_ps", bufs=1, space="PSUM") as fin_ps,
            tc.tile_pool(name="dram", bufs=1, space="DRAM") as dpool,
        ):
            ident = cpool.tile([128, 128], F32)
            make_identity(nc, ident[:])
            ident_bf = cpool.tile([128, 128], BF16)
            nc.vector.tensor_copy(ident_bf[:], ident[:])
            coff_sb = cpool.tile([128, 1], F32)
            nc.sync.dma_start(coff_sb[:], coff_d[:, :])
            lblg_sb = cpool.tile([128, G], I32)
            nc.sync.dma_start(lblg_sb[:], lblg_d[:, :])
            zidx = cpool.tile([128, 1], I32)
            nc.vector.memset(zidx[:], 0)
            ones_bf = cpool.tile([128, 1], BF16)
            nc.vector.memset(ones_bf[:], 1.0)

            # ---------------- embedding prep (high priority) ----------------
            # raw transposed embeddings stream in as bf16; 16/||e_b|| is
            # computed from the row-major copy, replicated across partitions
            # via a DRAM round-trip + index-0 gather, and multiplied in on DVE
            # producing the fp8 moving operand.
            e_sb = big.tile([128, G, D], F32)
            eT_sb = big.tile([128, KC, B], BF16)
            enT8 = big.tile([128, KC, B], FP8)
            es_sq = big.tile([128, G], F32)
            es_r = big.tile([128, G], F32)
            t_esr = dpool.tile([1, B], BF16)
            with tc.high_priority():
                nc.sync.dma_start(eT_sb[:], eT16_d[:, :, :])
                # ||e_b||^2 from the transposed copy: square on DVE, reduce
                # over the d partitions with a ones-matmul into a [1, B] row.
                eT2 = big.tile([128, KC, B], BF16)
                for k in range(KC):
                    nc.vector.tensor_mul(eT2[:, k, :], eT_sb[:, k, :], eT_sb[:, k, :])
                sqrow = tgt.tile([1, B], F32)
                for h in range(2):
                    cps = cos_ps.tile(
                        [128, 512], F32, tag=f"cos{h}", name=f"esq{h}",
                        bufs=3,
                    )
                    for k in range(KC):
                        nc.tensor.matmul(
                            cps[:1, :],
                            lhsT=ones_bf[:, 0:1],
                            rhs=eT2[:, k, h * 512:(h + 1) * 512],
                            start=(k == 0),
                            stop=(k == KC - 1),
                        )
                    nc.vector.tensor_copy(sqrow[:, h * 512:(h + 1) * 512], cps[:1, :])
                # 16/||e_b|| = rsqrt(ssq/256) on ScalarE (ssq > 0 so abs is free)
                esr_row16 = tgt.tile([1, B], BF16)
                nc.scalar.activation(
                    esr_row16[:], sqrow[:], AF.Abs_reciprocal_sqrt,
                    scale=1.0 / 256.0,
                )
                nc.sync.dma_start(t_esr[:, :], esr_row16[:])
                esrb = big.tile([128, B], BF16)
                nc.gpsimd.indirect_dma_start(
                    out=esrb[:],
                    out_offset=None,
                    in_=t_esr[:, :],
                    in_offset=bass.IndirectOffsetOnAxis(ap=zidx[:, 0:1], axis=0),
                )
                for k in range(KC):
                    nc.vector.tensor_mul(enT8[:, k, :], eT_sb[:, k, :], esrb[:])


            # ---------------- bulk weight prefetch ----------------
            # rows on HWDGE (bf16, for norms); lhsT tiles on SWDGE with a
            # bf16->fp8 cast in flight.  Only the first two chunks are issued
            # up front so they don't starve the embedding loads; the rest are
            # enqueued after the target path below.
            w_r = w16_d[:, :].rearrange("(t p) d -> p t d", p=128)
            rows_ch = []
            wT_ch = []

            def emit_chunk_dma(c):
                rt = wrow.tile([128, CH, D], BF16, tag="wr", name=f"wr{c}")
                nc.sync.dma_start(rt[:], w_r[:, c * CH:(c + 1) * CH])
                rows_ch.append(rt)
                wt = wtp.tile([128, KC, CH * 128], FP8, tag="wt", name=f"wt{c}")
                nc.gpsimd.dma_start(
                    wt[:], wT16_d[:, :, c * CH * 128:(c + 1) * CH * 128]
                )
                wT_ch.append(wt)

            for c in range(NCH):
                emit_chunk_dma(c)
            # row-major embeddings (target path only) load after the weights
            emb_r = emb_d[:, :].rearrange("(g p) d -> p g d", p=128)
            nc.sync.dma_start(e_sb[:], emb_r[:, :])

            # ---------------- per-class norms: scale pipeline ----------------
            # one fused square+row-sum per tile, emitted 14 tiles ahead of its
            # consumer so the DVE never bursts; Newton-rsqrt per chunk stays
            # one chunk ahead of the exp that reads it.
            ssq_all = big.tile([128, NT], F32)

            def emit_ssq_one(t):
                c, j = divmod(t, CH)
                sq = wrow.tile([128, D], BF16, tag="sq", bufs=2, name="sq")
                nc.vector.scalar_tensor_tensor(
                    out=sq[:], in0=rows_ch[c][:, j], scalar=1.0,
                    in1=rows_ch[c][:, j],
                    op0=ALU.mult, op1=ALU.mult,
                    accum_out=ssq_all[:, t:t + 1],
                )

            def emit_sc(c):
                # clamp: zero pad rows would otherwise overflow Newton to NaN
                scl = scp.tile([128, CH], F32, tag="scl", bufs=4, name=f"scl{c}")
                nc.vector.tensor_scalar_max(
                    scl[:], ssq_all[:, c * CH:(c + 1) * CH], 1.0
                )
                sc_c = scp.tile([128, CH], F32, tag="sc", bufs=4, name=f"sc{c}")
                _newton_rsqrt(nc, nrp, scl[:], sc_c[:], CH)
                nc.vector.tensor_scalar_mul(sc_c[:], sc_c[:], SCALE / 16.0)
                return sc_c

            for t in range(2 * CH):
                emit_ssq_one(t)
            sc_chunks = [emit_sc(0), emit_sc(1)]

            # ---------------- target-class path (early, overlaps main) -------
            lf = tgt.tile([128, G], F32)
            nc.vector.tensor_copy(lf[:], lblg_sb[:])          # i32 -> f32
            loc = tgt.tile([128, G], F32)
            nc.vector.tensor_scalar_sub(loc[:], lf[:], coff_sb[:])
            m1 = tgt.tile([128, G], F32)
            nc.vector.tensor_scalar(m1[:], loc[:], 0.0, None, op0=ALU.is_ge)
            m2 = tgt.tile([128, G], F32)
            nc.vector.tensor_scalar(m2[:], loc[:], float(CS), None, op0=ALU.is_lt)
            maskt = tgt.tile([128, G], F32)
            nc.vector.tensor_mul(maskt[:], m1[:], m2[:])
            locc = tgt.tile([128, G], F32)
            nc.vector.tensor_scalar_max(locc[:], loc[:], 0.0)
            nc.vector.tensor_scalar_min(locc[:], locc[:], float(CS - 1))
            loci = tgt.tile([128, G], I32)
            nc.vector.tensor_copy(loci[:], locc[:])            # f32 -> i32

            wt_g = big.tile([128, G, D], BF16)
            for g in range(G):
                nc.gpsimd.indirect_dma_start(
                    out=wt_g[:, g, :],
                    out_offset=None,
                    in_=w16_d[:, :],
                    in_offset=bass.IndirectOffsetOnAxis(ap=loci[:, g:g + 1], axis=0),
                )
            tssq = tgt.tile([128, G], F32)
            tdot = tgt.tile([128, G], F32)

            def emit_tnorm(g):
                scr = nrp.tile([128, D], F32, tag="escr", bufs=2, name="escr")
                nc.scalar.activation(
                    scr[:], e_sb[:, g], AF.Square, accum_out=es_sq[:, g:g + 1]
                )

            def emit_tstt(g):
                scr = nrp.tile([128, D], BF16, tag="tscr", bufs=2, name="tscr")
                nc.vector.scalar_tensor_tensor(
                    out=scr[:], in0=wt_g[:, g], scalar=1.0, in1=wt_g[:, g],
                    op0=ALU.mult, op1=ALU.mult, accum_out=tssq[:, g:g + 1],
                )
                scr2 = nrp.tile([128, D], F32, tag="tscr2", bufs=2, name="tscr2")
                nc.vector.scalar_tensor_tensor(
                    out=scr2[:], in0=e_sb[:, g], scalar=1.0, in1=wt_g[:, g],
                    op0=ALU.mult, op1=ALU.mult, accum_out=tdot[:, g:g + 1],
                )

            def emit_ct_chain():
                trs = tgt.tile([128, G], F32)
                _newton_rsqrt(nc, nrp, tssq[:], trs[:], G, name="tnr")
                ct = tgt.tile([128, G], F32)
                nc.vector.tensor_mul(ct[:], tdot[:], trs[:])
                nc.vector.tensor_mul(ct[:], ct[:], es_r[:])        # / ||e_b||
                t2 = tgt.tile([128, G], F32)
                nc.vector.tensor_mul(t2[:], ct[:], ct[:])
                nc.vector.tensor_scalar_min(t2[:], t2[:], 1.0)
                # sin = sqrt(1 - t2) = u * rsqrt(u), u = max(1 - t2, tiny)
                u = tgt.tile([128, G], F32)
                nc.vector.tensor_scalar(u[:], t2[:], -1.0, 1.0, op0=ALU.mult, op1=ALU.add)
                nc.vector.tensor_scalar_max(u[:], u[:], 1e-12)
                ur = tgt.tile([128, G], F32)
                _newton_rsqrt(nc, nrp, u[:], ur[:], G, name="unr")
                sint = tgt.tile([128, G], F32)
                nc.vector.tensor_mul(sint[:], u[:], ur[:])
                ctcm = tgt.tile([128, G], F32)
                nc.vector.tensor_scalar_mul(ctcm[:], ct[:], COS_M)
                phi = tgt.tile([128, G], F32)
                nc.vector.scalar_tensor_tensor(
                    out=phi[:], in0=sint[:], scalar=-SIN_M, in1=ctcm[:],
                    op0=ALU.mult, op1=ALU.add,
                )
                phif = tgt.tile([128, G], F32)
                nc.vector.tensor_scalar_sub(phif[:], ct[:], MM)
                cmp = tgt.tile([128, G], I32)
                nc.vector.tensor_scalar(cmp[:], ct[:], TH, None, op0=ALU.is_gt)
                nc.vector.copy_predicated(phif[:], cmp[:], phi[:])
                e1 = tgt.tile([128, G], F32)
                nc.scalar.activation(e1[:], phif[:], AF.Exp, scale=SCALE)
                e2 = tgt.tile([128, G], F32)
                nc.scalar.activation(e2[:], ct[:], AF.Exp, scale=SCALE)
                dd = tgt.tile([128, G], F32)
                nc.vector.tensor_sub(dd[:], e1[:], e2[:])
                nc.vector.tensor_mul(dd[:], dd[:], maskt[:])
                pp = tgt.tile([128, G], F32)
                nc.vector.tensor_mul(pp[:], phif[:], maskt[:])

                # stage dd/pp into the collective input (one AllReduce at end)
                nc.sync.dma_start(ar_in[1], dd[:])
                nc.sync.dma_start(ar_in[2], pp[:])


            ar_in = dpool.tile([3, 128, G], F32)
            ar_out = dpool.tile([3, 128, G], F32, addr_space="Shared")
            # ---------------- main class-tile loop (fp8 DoubleRow) ----------
            acc0 = big.tile([128, B], BF16)
            acc1 = big.tile([128, B], BF16)
            acc2 = big.tile([128, B], BF16)
            nc.vector.memset(acc0[:], 0.0)
            nc.vector.memset(acc1[:], 0.0)
            nc.vector.memset(acc2[:], 0.0)
            for t in range(NT):
                c, j = divmod(t, CH)
                wTc = wT_ch[c]
                ex = exp_pool.tile([128, B], BF16, tag="ex", name="ex", bufs=6)
                for h in range(2):
                    cps = cos_ps.tile(
                        [128, 512], F32, tag=f"cos{h}", name=f"cps{h}",
                        bufs=3,
                    )
                    for jj in range(2):
                        nc.tensor.matmul(
                            cps[:],
                            lhsT=wTc[:, 2 * jj:2 * jj + 2, j * 128:(j + 1) * 128],
                            rhs=enT8[:, 2 * jj:2 * jj + 2, h * 512:(h + 1) * 512],
                            start=(jj == 0),
                            stop=(jj == 1),
                            perf_mode=DR,
                        )
                    nc.scalar.activation(
                        ex[:, h * 512:(h + 1) * 512], cps[:], AF.Exp,
                        scale=sc_chunks[c][:, j:j + 1],
                    )
                a = (acc0, acc1, acc2)[t % 3]
                nc.vector.tensor_add(a[:], a[:], ex[:])
                if t + 2 * CH < NT:
                    emit_ssq_one(t + 2 * CH)
                if j == CH - 1 and c + 2 < NCH:
                    sc_chunks.append(emit_sc(c + 2))
                if t >= 8 and t <= 22 and t % 2 == 0:
                    emit_tnorm((t - 8) // 2)
                if t == 24:
                    _newton_rsqrt(nc, nrp, es_sq[:], es_r[:], G, name="enr")
                if t >= 16 and t <= 37 and (t - 16) % 3 == 0:
                    emit_tstt((t - 16) // 3)
                if t == 44:
                    emit_ct_chain()


            # ---------------- reduce partials over class partitions ---------
            nc.vector.tensor_add(acc0[:], acc0[:], acc1[:])
            nc.vector.tensor_add(acc0[:], acc0[:], acc2[:])
            S_sb = tgt.tile([128, G], F32)
            for g in range(G):
                ptg = pt_ps.tile([128, 128], BF16, tag="ptb", name="ptg", bufs=1)
                nc.tensor.transpose(
                    ptg[:], acc0[:, g * 128:(g + 1) * 128], ident_bf[:]
                )
                nc.vector.reduce_sum(S_sb[:, g:g + 1], ptg[:], axis=AX.X)

            # ---------------- final AllReduce of [S, dd, pp] ----------------
            nc.sync.dma_start(ar_in[0], S_sb[:])
            nc.gpsimd.collective_compute(
                "AllReduce",
                ALU.add,
                replica_groups=[list(range(NCORES))],
                ins=[ar_in[:].opt()],
                outs=[ar_out[:].opt()],
            )
            SDP = tgt.tile([128, 3, G], F32)
            nc.sync.dma_start(SDP[:], ar_out[:, :, :].rearrange("t p g -> p t g"))

            # ---------------- finale ----------------
            Zt = tgt.tile([128, G], F32)
            nc.vector.tensor_add(Zt[:], SDP[:, 0], SDP[:, 1])
            nc.vector.tensor_scalar_sub(Zt[:], Zt[:], PADS)
            lnz = tgt.tile([128, G], F32)
            nc.scalar.activation(lnz[:], Zt[:], AF.Ln)
            nll = tgt.tile([128, G], F32)
            nc.vector.scalar_tensor_tensor(
                out=nll[:], in0=SDP[:, 2], scalar=-SCALE, in1=lnz[:],
                op0=ALU.mult, op1=ALU.add,
            )
            csum = tgt.tile([128, 1], F32)
            nc.vector.reduce_sum(csum[:], nll[:], axis=AX.X)
            ptf = fin_ps.tile([128, 128], F32, tag="fin", name="ptf", bufs=1)
            nc.tensor.transpose(ptf[:1, :], csum[:], ident[:])
            fin = tgt.tile([1, 1], F32)
            nc.vector.tensor_reduce(fin[:], ptf[:1, :], axis=AX.X, op=ALU.add)
            nc.vector.tensor_scalar_mul(fin[:], fin[:], 1.0 / B)
            nc.sync.dma_start(out_d[:, :], fin[:])

    nc.compile()
    return nc


def _shard_inputs(embeddings, labels, weight):
    import ml_dtypes

    emb = np.ascontiguousarray(embeddings, dtype=np.float32)
    lbl = np.ascontiguousarray(labels, dtype=np.int32)
    w = np.asarray(weight, dtype=np.float32)
    # labels in g-blocked layout: lblg[p, g] = labels[g*128 + p]
    lblg = np.ascontiguousarray(lbl.reshape(G, 128).T)
    # raw transposed embeddings: eT16[p, k, b] = emb[b, 128k + p]
    eT16 = np.ascontiguousarray(
        emb.T.reshape(KC, 128, B).transpose(1, 0, 2).astype(ml_dtypes.bfloat16)
    )
    in_maps = []
    for i in range(NCORES):
        w16 = np.zeros((CP, D), ml_dtypes.bfloat16)
        w16[:CS] = (16.0 * w[i * CS:(i + 1) * CS]).astype(ml_dtypes.bfloat16)
        # lhsT layout [128, KC, CP]: wT16[p, k, c] = w16[c, 128k + p]
        wT16 = np.ascontiguousarray(
            w16.T.reshape(KC, 128, CP).transpose(1, 0, 2)
        )
        coff = np.full((128, 1), i * CS, np.float32)
        in_maps.append(
            {
                "emb": emb,
                "eT16": eT16,
                "w16": w16,
                "wT16": wT16,
                "lblg": lblg,
                "coff": coff,
            }
        )
    return in_maps


def kernel(embeddings, labels, weight):
    global _CACHED_NC
    if _CACHED_NC is None:
        _CACHED_NC = build_kernel()
    in_maps = _shard_inputs(embeddings, labels, weight)
    res = run_bass_kernel_spmd(_CACHED_NC, in_maps, core_ids=list(range(NCORES)))
    return np.float32(res.results[0]["out"][0, 0])


if __name__ == "__main__":
    rng = np.random.default_rng(0)
    emb = rng.standard_normal((B, D), dtype=np.float32)
    lbl = rng.integers(0, C, size=(B,), dtype=np.int32)
    w = (rng.random((C, D), dtype=np.float32) - 0.5) * 0.02
    print("loss =", kernel(emb, lbl, w))

